# revision 6
# baseline (speedup 1.0000x reference)
"""Trainium2 Bass kernel for nn_Attention_msa (sparse attention, 8-core SPMD).

Sharding: query rows split across 8 cores (250 rows each), K/V replicated.
Per-core inputs are rotated along the position axis by -250*core so the
block-diagonal mask strip sits at a compile-time-constant column range and a
single compiled program serves all cores. No collectives.
"""

import numpy as np

import concourse.bass as bass
import concourse.bacc as bacc
import concourse.mybir as mybir
import concourse.tile as tile
from concourse.bass_utils import run_bass_kernel_spmd
from concourse.masks import make_identity

F32 = mybir.dt.float32
BF16 = mybir.dt.bfloat16
AX = mybir.AxisListType
OP = mybir.AluOpType
AF = mybir.ActivationFunctionType

N = 2000
NP = 2048  # padded positions (16 tiles of 128)
C = 256
H = 8
HD = 32
NCORES = 8
R = 250          # rows per core
RT = 125         # rows per rowtile
NEG = -30000.0   # pad-score fill; exp(scale*NEG) == 0


def _build_program():
    nc = bacc.Bacc("TRN2", target_bir_lowering=False, debug=False,
                   num_devices=NCORES)

    d_xc = nc.dram_tensor("xc", [NP, C], F32, kind="ExternalInput").ap()
    d_xr = nc.dram_tensor("xr", [NP, C], F32, kind="ExternalInput").ap()
    d_wc = nc.dram_tensor("wc", [C, 3 * C], F32, kind="ExternalInput").ap()
    d_wr = nc.dram_tensor("wr", [C, 3 * C], F32, kind="ExternalInput").ap()
    d_bi4 = nc.dram_tensor("bi4", [128, 4], F32, kind="ExternalInput").ap()
    d_mst = nc.dram_tensor("mst", [2, RT, 130], F32, kind="ExternalInput").ap()
    d_xout = nc.dram_tensor("xout", [R, 2 * C], F32, kind="ExternalOutput").ap()
    d_sim = nc.dram_tensor("simout", [R, N], F32, kind="ExternalOutput").ap()

    # DRAM scratch for partition-reshape bounces (per tensor-chunk)
    scr_a = [nc.dram_tensor(f"scra{i}", [4, 16, 128], F32).ap() for i in range(6)]
    scr_b = [nc.dram_tensor(f"scrb{i}", [4, 16, 128], F32).ap() for i in range(6)]

    with tile.TileContext(nc) as tc:
        with tc.tile_pool(name="persist", bufs=1) as pp:
            ident = pp.tile([128, 128], F32)
            make_identity(nc, ident[:])
            bi4 = pp.tile([128, 4], F32)
            nc.sync.dma_start(out=bi4[:], in_=d_bi4[:])
            mst = pp.tile([128, 2, 130], F32)
            nc.sync.dma_start(out=mst[0:RT, :, :],
                              in_=d_mst.rearrange("t p s -> p t s"))

            # persistent big tensors
            kTc = [pp.tile([128, NP], F32, tag=f"kTc{i}", name=f"kTc{i}") for i in range(2)]
            kTr = [pp.tile([128, NP], F32, tag=f"kTr{i}", name=f"kTr{i}") for i in range(2)]
            vT = [pp.tile([128, NP], F32, tag=f"vT{i}", name=f"vT{i}") for i in range(2)]
            qTc = [pp.tile([128, R], F32, tag=f"qTc{i}", name=f"qTc{i}") for i in range(2)]
            qTr = [pp.tile([128, R], F32, tag=f"qTr{i}", name=f"qTr{i}") for i in range(2)]
            v_nat = pp.tile([128, 16, C], F32)
            v_bf = pp.tile([128, 16, C], BF16)
            inq = pp.tile([128, 32], F32)   # [:, s*16 + t*8 + h] = 25/|q|
            smask = [pp.tile([128, NP], F32, tag=f"sm{i}", name=f"sm{i}") for i in range(2)]

            # ---------------- Phase A+B: x transpose + QKV ----------------
            with tc.tile_pool(name="stage", bufs=1) as sp, \
                 tc.tile_pool(name="pst", bufs=2, space="PSUM") as pst:
                w_sb = {}
                for sname, dw in (("c", d_wc), ("r", d_wr)):
                    for kk in range(2):
                        w = sp.tile([128, 3 * C], F32, tag=f"w{sname}{kk}")
                        nc.sync.dma_start(out=w[:],
                                          in_=dw[kk * 128:(kk + 1) * 128, :])
                        w_sb[(sname, kk)] = w
                xT = {}
                for sname, dx in (("c", d_xc), ("r", d_xr)):
                    xnat = sp.tile([128, 16, C], F32, tag=f"xn{sname}")
                    nc.sync.dma_start(
                        out=xnat[:],
                        in_=dx.rearrange("(j p) c -> p j c", p=128))
                    for kk in range(2):
                        xt = sp.tile([128, NP], F32, tag=f"xT{sname}{kk}")
                        xT[(sname, kk)] = xt
                        for j in range(16):
                            ps = pst.tile([128, 128], F32, tag="tp")
                            nc.tensor.transpose(
                                ps[:], xnat[:, j, kk * 128:(kk + 1) * 128],
                                ident[:])
                            nc.vector.tensor_copy(
                                xt[:, j * 128:(j + 1) * 128], ps[:])

                # QKV matmuls (fp32 exact)
                def mm_to(dst, sname, col0, ncols):
                    # dst: list of 2 sbuf chunk tiles [128, ncols-wide...]
                    for cc in range(2):
                        nch = (ncols + 511) // 512
                        for nn in range(nch):
                            w0 = nn * 512
                            w1 = min(ncols, w0 + 512)
                            ps = pst.tile([128, 512], F32, tag="qkv")
                            for kk in range(2):
                                nc.tensor.matmul(
                                    ps[:, 0:w1 - w0],
                                    w_sb[(sname, kk)][:, col0 + cc * 128:
                                                      col0 + cc * 128 + 128],
                                    xT[(sname, kk)][:, w0:w1],
                                    start=(kk == 0), stop=(kk == 1))
                            eng = nc.vector if nn % 2 == 0 else nc.scalar
                            if eng is nc.scalar:
                                nc.scalar.copy(dst[cc][:, w0:w1],
                                               ps[:, 0:w1 - w0])
                            else:
                                nc.vector.tensor_copy(dst[cc][:, w0:w1],
                                                      ps[:, 0:w1 - w0])

                mm_to(kTc, "c", C, NP)
                mm_to(kTr, "r", C, NP)
                mm_to(vT, "c", 2 * C, NP)
                mm_to(qTc, "c", 0, R)
                mm_to(qTr, "r", 0, R)
                # v natural [m, c] tiles
                for j in range(16):
                    ps = pst.tile([128, C], F32, tag="vn")
                    for kk in range(2):
                        nc.tensor.matmul(
                            ps[:], xT[("c", kk)][:, j * 128:(j + 1) * 128],
                            w_sb[("c", kk)][:, 2 * C:3 * C],
                            start=(kk == 0), stop=(kk == 1))
                    nc.vector.tensor_copy(v_nat[:, j, :], ps[:])
                for half in range(2):
                    eng = nc.vector if half == 0 else nc.gpsimd
                    eng.tensor_copy(v_bf[:, half * 8:(half + 1) * 8, :],
                                    v_nat[:, half * 8:(half + 1) * 8, :])

            # ---------------- Phase C: norms ----------------
            with tc.tile_pool(name="nrm", bufs=2) as np_, \
                 tc.tile_pool(name="nps", bufs=1, space="PSUM") as nps:
                si = 0
                for tens in (kTc, kTr, vT):
                    for cc in range(2):
                        sq = np_.tile([128, NP], F32, tag="sq")
                        eng = nc.vector if cc == 0 else nc.gpsimd
                        eng.tensor_tensor(
                            out=sq[:], in0=tens[cc][:], in1=tens[cc][:],
                            op=OP.mult)
                        ss = nps.tile([4, 4, 512], F32, tag="ss")
                        for nn in range(4):
                            nc.tensor.matmul(
                                ss[:, nn, :], bi4[:],
                                sq[:, nn * 512:(nn + 1) * 512],
                                start=True, stop=True)
                        sn = np_.tile([4, NP], F32, tag="sn")
                        nc.scalar.sqrt(
                            sn[:].rearrange("p (a b) -> p a b", a=4), ss[:])
                        # bounce reshape: [4, 2048] -> [128, 64]
                        nc.sync.dma_start(
                            out=scr_a[si].rearrange("h a p -> h (a p)"),
                            in_=sn[:])
                        rsh = np_.tile([128, 64], F32, tag="rsh")
                        nc.sync.dma_start(
                            out=rsh[:],
                            in_=scr_a[si].rearrange("h a p -> p (h a)"))
                        rin = np_.tile([128, 64], F32, tag="rin")
                        nc.vector.reciprocal(rin[:], rsh[:])
                        nc.sync.dma_start(
                            out=scr_b[si].rearrange("h a p -> p (h a)"),
                            in_=rin[:])
                        inkb = np_.tile([128, NP], F32, tag="inkb")
                        for h4 in range(4):
                            bc = bass.AP(
                                tensor=scr_b[si].tensor,
                                offset=h4 * NP,
                                ap=[[0, 32], [1, NP]])
                            nc.sync.dma_start(
                                out=inkb[h4 * 32:(h4 + 1) * 32, :], in_=bc)
                        eng = nc.vector if cc == 1 else nc.gpsimd
                        eng.tensor_tensor(out=tens[cc][:], in0=tens[cc][:],
                                          in1=inkb[:], op=OP.mult)
                        si += 1

                # q norms -> inq (25/|q| per row), via PE transpose of [4,125]
                for s_i, qT in enumerate((qTc, qTr)):
                    for cc in range(2):
                        sq = np_.tile([128, R], F32, tag="sqq")
                        nc.vector.tensor_tensor(out=sq[:], in0=qT[cc][:],
                                                in1=qT[cc][:], op=OP.mult)
                        ssq = nps.tile([4, R], F32, tag="ssq")
                        nc.tensor.matmul(ssq[:], bi4[:], sq[:],
                                         start=True, stop=True)
                        snq = np_.tile([4, R], F32, tag="snq")
                        # sqrt(ss/625) = |q|/25
                        nc.scalar.activation(snq[:], ssq[:], AF.Sqrt,
                                             scale=1.0 / 625.0)
                        for t in range(2):
                            tp = nps.tile([128, 4], F32, tag="tq")
                            nc.tensor.transpose(
                                tp[0:RT, :], snq[:, t * RT:(t + 1) * RT],
                                ident[0:4, 0:4])
                            nc.vector.tensor_copy(
                                inq[0:RT, s_i * 16 + t * 8 + cc * 4:
                                    s_i * 16 + t * 8 + cc * 4 + 4],
                                tp[0:RT, :])
                nc.vector.reciprocal(inq[0:RT, :], inq[0:RT, :])

            # ---------------- Phase D: vv scores -> sim mask ----------------
            with tc.tile_pool(name="vvp", bufs=2, space="PSUM") as vvp:
                for t in range(2):
                    raw = vvp.tile([128, 4, 512], F32, tag="raw")
                    for nn in range(4):
                        for kk in range(2):
                            nc.tensor.matmul(
                                raw[0:RT, nn, :],
                                vT[kk][:, t * RT:t * RT + RT],
                                vT[kk][:, nn * 512:(nn + 1) * 512],
                                start=(kk == 0), stop=(kk == 1))
                    nc.vector.tensor_scalar(
                        out=smask[t][0:RT, :].rearrange("p (a b) -> p a b",
                                                        a=4),
                        in0=raw[0:RT, :, :], scalar1=6.0, scalar2=None,
                        op0=OP.is_gt)

            # ---------------- Phase E: attention ----------------
            with tc.tile_pool(name="att", bufs=2) as ap_, \
                 tc.tile_pool(name="scp", bufs=2, space="PSUM") as scp, \
                 tc.tile_pool(name="xup", bufs=2, space="PSUM") as xup:
                for t in range(2):
                    sim_t = ap_.tile([128, NP], F32, tag="sim")
                    icr = ap_.tile([128, 8], F32, tag="icr")
                    xu = [xup.tile([128, 128], F32, tag="xu", name="xu") for _ in range(2)]
                    for h in range(8):
                        cc, hh = h // 4, h % 4
                        b0 = 32 * hh
                        eC = ap_.tile([128, NP], F32, tag="eC")
                        eR = ap_.tile([128, NP], F32, tag="eR")
                        den = ap_.tile([128, 4], F32, tag="den")
                        for mi, (kT, qT, e_t, dof) in enumerate(
                                ((kTc, qTc, eC, 0), (kTr, qTr, eR, 2))):
                            for half in range(2):
                                sc = scp.tile([128, 2, 512], F32, tag="sc")
                                for nn in range(2):
                                    m0 = (half * 2 + nn) * 512
                                    nc.tensor.matmul(
                                        sc[0:RT, nn, :],
                                        qT[cc][b0:b0 + 32, t * RT:t * RT + RT],
                                        kT[cc][b0:b0 + 32, m0:m0 + 512],
                                        start=True, stop=True,
                                        tile_position=(b0, 0))
                                if half == 1:
                                    nc.vector.memset(
                                        sc[0:RT, 1, N - 1536:512], NEG)
                                nc.scalar.activation(
                                    e_t[0:RT, half * 1024:(half + 1) * 1024]
                                    .rearrange("p (a b) -> p a b", a=2),
                                    sc[0:RT, :, :], AF.Exp,
                                    scale=inq[0:RT, mi * 16 + t * 8 + h:
                                              mi * 16 + t * 8 + h + 1],
                                    accum_out=den[0:RT, dof + half:
                                                  dof + half + 1])
                        # denominators -> rho, icR2
                        dsum = ap_.tile([128, 2], F32, tag="dsum")
                        nc.vector.tensor_tensor(out=dsum[0:RT, 0:1],
                                                in0=den[0:RT, 0:1],
                                                in1=den[0:RT, 1:2], op=OP.add)
                        nc.vector.tensor_tensor(out=dsum[0:RT, 1:2],
                                                in0=den[0:RT, 2:3],
                                                in1=den[0:RT, 3:4], op=OP.add)
                        rden = ap_.tile([128, 2], F32, tag="rden")
                        nc.vector.reciprocal(rden[0:RT, :], dsum[0:RT, :])
                        rho = ap_.tile([128, 1], F32, tag="rho")
                        nc.vector.tensor_tensor(out=rho[0:RT, :],
                                                in0=dsum[0:RT, 1:2],
                                                in1=rden[0:RT, 0:1],
                                                op=OP.mult)
                        nc.vector.tensor_scalar(
                            out=icr[0:RT, h:h + 1], in0=rden[0:RT, 1:2],
                            scalar1=0.5, scalar2=None, op0=OP.mult)
                        # u = rho*eC + eR  (bf16)
                        u_bf = ap_.tile([128, NP], BF16, tag="u")
                        nc.vector.scalar_tensor_tensor(
                            out=u_bf[0:RT, :], in0=eC[0:RT, :],
                            scalar=rho[0:RT, :], in1=eR[0:RT, :],
                            op0=OP.mult, op1=OP.add)
                        # block mask strip
                        st = 0 if t == 0 else 120
                        nc.vector.tensor_tensor(
                            out=u_bf[0:RT, st:st + 130],
                            in0=u_bf[0:RT, st:st + 130],
                            in1=mst[0:RT, t, :], op=OP.mult)
                        # sim += icR2 * u
                        seng = nc.vector
                        if h == 0:
                            seng.tensor_scalar(
                                out=sim_t[0:RT, :], in0=u_bf[0:RT, :],
                                scalar1=icr[0:RT, h:h + 1], scalar2=None,
                                op0=OP.mult)
                        else:
                            seng.scalar_tensor_tensor(
                                out=sim_t[0:RT, :], in0=u_bf[0:RT, :],
                                scalar=icr[0:RT, h:h + 1], in1=sim_t[0:RT, :],
                                op0=OP.mult, op1=OP.add)
                        # transpose u via DMA xbar, then attn @ v
                        uT = ap_.tile([128, 16, 128], BF16, tag="uT")
                        nc.sync.dma_start_transpose(out=uT[:], in_=u_bf[:])
                        for j in range(16):
                            nc.tensor.matmul(
                                xu[cc][b0:b0 + 32, 0:RT],
                                v_bf[:, j, h * 32:h * 32 + 32],
                                uT[:, j, 0:RT],
                                start=(j == 0), stop=(j == 15),
                                tile_position=(0, b0))
                    # finalize x for this rowtile
                    x_sb = ap_.tile([128, C], F32, tag="xsb")
                    for cc in range(2):
                        xs = ap_.tile([128, 128], F32, tag="xs")
                        nc.vector.tensor_copy(xs[:, 0:RT], xu[cc][:, 0:RT])
                        xt = xup.tile([128, 128], F32, tag="xt")
                        nc.tensor.transpose(xt[0:RT, :], xs[:, 0:RT].rearrange(
                            "p r -> p r"), ident[:])
                        for hh in range(4):
                            h = cc * 4 + hh
                            nc.vector.tensor_scalar(
                                out=x_sb[0:RT, h * 32:h * 32 + 32],
                                in0=xt[0:RT, hh * 32:hh * 32 + 32],
                                scalar1=icr[0:RT, h:h + 1], scalar2=None,
                                op0=OP.mult)
                    nc.sync.dma_start(out=d_xout[t * RT:(t + 1) * RT, 0:C],
                                      in_=x_sb[0:RT, :])
                    # finalize sim_round2 for this rowtile
                    es = ap_.tile([128, NP], F32, tag="esm", name="es")
                    d1 = ap_.tile([128, 1], F32, tag="d1")
                    nc.scalar.activation(es[0:RT, :], sim_t[0:RT, :], AF.Exp,
                                         accum_out=d1[0:RT, :])
                    em = ap_.tile([128, NP], F32, tag="esm", name="em")
                    s_ = ap_.tile([128, 1], F32, tag="s_")
                    nc.vector.scalar_tensor_tensor(
                        out=em[0:RT, :], in0=es[0:RT, :], scalar=1.0,
                        in1=smask[t][0:RT, :], op0=OP.mult, op1=OP.mult,
                        accum_out=s_[0:RT, :])
                    dd = ap_.tile([128, 1], F32, tag="dd")
                    nc.vector.scalar_tensor_tensor(
                        out=dd[0:RT, :], in0=d1[0:RT, :], scalar=1e-8,
                        in1=s_[0:RT, :], op0=OP.mult, op1=OP.add)
                    rdd = ap_.tile([128, 1], F32, tag="rdd")
                    nc.vector.reciprocal(rdd[0:RT, :], dd[0:RT, :])
                    simo = ap_.tile([128, NP], F32, tag="esm", name="simo")
                    seng2 = nc.vector
                    seng2.tensor_scalar(out=simo[0:RT, :], in0=em[0:RT, :],
                                        scalar1=rdd[0:RT, :], scalar2=None,
                                        op0=OP.mult)
                    nc.sync.dma_start(out=d_sim[t * RT:(t + 1) * RT, :],
                                      in_=simo[0:RT, 0:N])
                # x_ori columns straight from v_nat rows 0..250
                nc.sync.dma_start(out=d_xout[0:128, C:2 * C],
                                  in_=v_nat[:, 0, :])
                nc.sync.dma_start(out=d_xout[128:R, C:2 * C],
                                  in_=v_nat[0:R - 128, 1, :])

    nc.compile()
    return nc


_NC_CACHE = None


def kernel(x_cls, x_reg, W_qkv_cls, W_qkv_reg):
    global _NC_CACHE
    x_cls = np.asarray(x_cls)
    x_reg = np.asarray(x_reg)
    W_qkv_cls = np.ascontiguousarray(np.asarray(W_qkv_cls), dtype=np.float32)
    W_qkv_reg = np.ascontiguousarray(np.asarray(W_qkv_reg), dtype=np.float32)
    xc0 = x_cls[0].astype(np.float32)
    xr0 = x_reg[0].astype(np.float32)

    bi4 = np.zeros((128, 4), dtype=np.float32)
    for h4 in range(4):
        bi4[h4 * 32:(h4 + 1) * 32, h4] = 1.0

    # mask strip (core-independent thanks to rotation; 250 % 10 == 0)
    mst = np.ones((2, RT, 130), dtype=np.float32)
    strip0 = [0, 120]
    for t in range(2):
        for i in range(RT):
            row = t * RT + i
            blk = (row // 10) * 10
            for j in range(blk, blk + 9):
                if j != row:
                    mst[t, i, j - strip0[t]] = 0.0

    if _NC_CACHE is None:
        _NC_CACHE = _build_program()
    nc = _NC_CACHE

    in_maps = []
    for core in range(NCORES):
        r0 = R * core
        xc = np.roll(xc0, -r0, axis=0)
        xr = np.roll(xr0, -r0, axis=0)
        xcp = np.zeros((NP, C), dtype=np.float32)
        xcp[0:N] = xc
        xrp = np.zeros((NP, C), dtype=np.float32)
        xrp[0:N] = xr
        in_maps.append({
            "xc": xcp, "xr": xrp, "wc": W_qkv_cls, "wr": W_qkv_reg,
            "bi4": bi4, "mst": mst,
        })

    res = run_bass_kernel_spmd(nc, in_maps, core_ids=list(range(NCORES)))

    x_out = np.zeros((1, N, 2 * C), dtype=np.float32)
    sim = np.zeros((N, N), dtype=np.float32)
    for core in range(NCORES):
        r0 = R * core
        x_out[0, r0:r0 + R, :] = res.results[core]["xout"]
        sim[r0:r0 + R, :] = np.roll(res.results[core]["simout"], r0, axis=1)
    return (x_out, sim)


if __name__ == "__main__":
    import reference
    ins = reference.setup_inputs()
    out = kernel(**{k: np.asarray(v) for k, v in ins.items()})
    print("ok", out[0].shape, out[1].shape)


# revision 7
# speedup vs baseline: 1.0160x; 1.0160x over previous
"""Trainium2 Bass kernel for nn_Attention_msa (sparse attention, 8-core SPMD).

Sharding: query rows split across 8 cores (250 rows each), K/V replicated.
Per-core inputs are rotated along the position axis by -250*core so the
block-diagonal mask strip sits at a compile-time-constant column range and a
single compiled program serves all cores. No collectives.
"""

import numpy as np

import concourse.bass as bass
import concourse.bacc as bacc
import concourse.mybir as mybir
import concourse.tile as tile
from concourse.bass_utils import run_bass_kernel_spmd
from concourse.masks import make_identity

F32 = mybir.dt.float32
BF16 = mybir.dt.bfloat16
AX = mybir.AxisListType
OP = mybir.AluOpType
AF = mybir.ActivationFunctionType

N = 2000
NP = 2048  # padded positions (16 tiles of 128)
C = 256
H = 8
HD = 32
NCORES = 8
R = 250          # rows per core
RT = 125         # rows per rowtile
NEG = -30000.0   # pad-score fill; exp(scale*NEG) == 0


def _build_program():
    nc = bacc.Bacc("TRN2", target_bir_lowering=False, debug=False,
                   num_devices=NCORES)

    d_xc = nc.dram_tensor("xc", [NP, C], F32, kind="ExternalInput").ap()
    d_xr = nc.dram_tensor("xr", [NP, C], F32, kind="ExternalInput").ap()
    d_wc = nc.dram_tensor("wc", [C, 3 * C], F32, kind="ExternalInput").ap()
    d_wr = nc.dram_tensor("wr", [C, 3 * C], F32, kind="ExternalInput").ap()
    d_bi4 = nc.dram_tensor("bi4", [128, 4], F32, kind="ExternalInput").ap()
    d_mst = nc.dram_tensor("mst", [2, RT, 130], F32, kind="ExternalInput").ap()
    d_xout = nc.dram_tensor("xout", [R, 2 * C], F32, kind="ExternalOutput").ap()
    d_sim = nc.dram_tensor("simout", [R, N], F32, kind="ExternalOutput").ap()

    # DRAM scratch for partition-reshape bounces (per tensor-chunk)
    scr_a = [nc.dram_tensor(f"scra{i}", [4, 16, 128], F32).ap() for i in range(6)]
    scr_b = [nc.dram_tensor(f"scrb{i}", [4, 16, 128], F32).ap() for i in range(6)]

    with tile.TileContext(nc) as tc:
        with tc.tile_pool(name="persist", bufs=1) as pp:
            ident = pp.tile([128, 128], F32)
            make_identity(nc, ident[:])
            bi4 = pp.tile([128, 4], F32)
            nc.sync.dma_start(out=bi4[:], in_=d_bi4[:])
            mst = pp.tile([128, 2, 130], F32)
            nc.sync.dma_start(out=mst[0:RT, :, :],
                              in_=d_mst.rearrange("t p s -> p t s"))

            # persistent big tensors
            kTc = [pp.tile([128, NP], F32, tag=f"kTc{i}", name=f"kTc{i}") for i in range(2)]
            kTr = [pp.tile([128, NP], F32, tag=f"kTr{i}", name=f"kTr{i}") for i in range(2)]
            vT = [pp.tile([128, NP], F32, tag=f"vT{i}", name=f"vT{i}") for i in range(2)]
            qTc = [pp.tile([128, R], F32, tag=f"qTc{i}", name=f"qTc{i}") for i in range(2)]
            qTr = [pp.tile([128, R], F32, tag=f"qTr{i}", name=f"qTr{i}") for i in range(2)]
            v_nat = pp.tile([128, 16, C], F32)
            v_bf = pp.tile([128, 16, C], BF16)
            inq = pp.tile([128, 32], F32)   # [:, s*16 + t*8 + h] = 25/|q|
            smask = [pp.tile([128, NP], F32, tag=f"sm{i}", name=f"sm{i}") for i in range(2)]

            # ---------------- Phase A+B: x transpose + QKV ----------------
            with tc.tile_pool(name="stage", bufs=1) as sp, \
                 tc.tile_pool(name="pst", bufs=2, space="PSUM") as pst:
                w_sb = {}
                for sname, dw in (("c", d_wc), ("r", d_wr)):
                    for kk in range(2):
                        w = sp.tile([128, 3 * C], F32, tag=f"w{sname}{kk}")
                        nc.sync.dma_start(out=w[:],
                                          in_=dw[kk * 128:(kk + 1) * 128, :])
                        w_sb[(sname, kk)] = w
                xT = {}
                for sname, dx in (("c", d_xc), ("r", d_xr)):
                    xnat = sp.tile([128, 16, C], F32, tag=f"xn{sname}")
                    nc.sync.dma_start(
                        out=xnat[:],
                        in_=dx.rearrange("(j p) c -> p j c", p=128))
                    for kk in range(2):
                        xt = sp.tile([128, NP], F32, tag=f"xT{sname}{kk}")
                        xT[(sname, kk)] = xt
                        for j in range(16):
                            ps = pst.tile([128, 128], F32, tag="tp")
                            nc.tensor.transpose(
                                ps[:], xnat[:, j, kk * 128:(kk + 1) * 128],
                                ident[:])
                            nc.vector.tensor_copy(
                                xt[:, j * 128:(j + 1) * 128], ps[:])

                # QKV matmuls (fp32 exact)
                def mm_to(dst, sname, col0, ncols):
                    # dst: list of 2 sbuf chunk tiles [128, ncols-wide...]
                    for cc in range(2):
                        nch = (ncols + 511) // 512
                        for nn in range(nch):
                            w0 = nn * 512
                            w1 = min(ncols, w0 + 512)
                            ps = pst.tile([128, 512], F32, tag="qkv")
                            for kk in range(2):
                                nc.tensor.matmul(
                                    ps[:, 0:w1 - w0],
                                    w_sb[(sname, kk)][:, col0 + cc * 128:
                                                      col0 + cc * 128 + 128],
                                    xT[(sname, kk)][:, w0:w1],
                                    start=(kk == 0), stop=(kk == 1))
                            eng = nc.vector if nn % 2 == 0 else nc.scalar
                            if eng is nc.scalar:
                                nc.scalar.copy(dst[cc][:, w0:w1],
                                               ps[:, 0:w1 - w0])
                            else:
                                nc.vector.tensor_copy(dst[cc][:, w0:w1],
                                                      ps[:, 0:w1 - w0])

                mm_to(kTc, "c", C, NP)
                mm_to(kTr, "r", C, NP)
                mm_to(vT, "c", 2 * C, NP)
                mm_to(qTc, "c", 0, R)
                mm_to(qTr, "r", 0, R)
                # v natural [m, c] tiles
                for j in range(16):
                    ps = pst.tile([128, C], F32, tag="vn")
                    for kk in range(2):
                        nc.tensor.matmul(
                            ps[:], xT[("c", kk)][:, j * 128:(j + 1) * 128],
                            w_sb[("c", kk)][:, 2 * C:3 * C],
                            start=(kk == 0), stop=(kk == 1))
                    nc.vector.tensor_copy(v_nat[:, j, :], ps[:])
                for half in range(2):
                    eng = nc.vector if half == 0 else nc.gpsimd
                    eng.tensor_copy(v_bf[:, half * 8:(half + 1) * 8, :],
                                    v_nat[:, half * 8:(half + 1) * 8, :])

            # ---------------- Phase C: norms ----------------
            with tc.tile_pool(name="nrm", bufs=2) as np_, \
                 tc.tile_pool(name="nps", bufs=1, space="PSUM") as nps:
                si = 0
                for tens in (kTc, kTr, vT):
                    for cc in range(2):
                        sq = np_.tile([128, NP], F32, tag="sq")
                        eng = nc.vector if cc == 0 else nc.gpsimd
                        eng.tensor_tensor(
                            out=sq[:], in0=tens[cc][:], in1=tens[cc][:],
                            op=OP.mult)
                        ss = nps.tile([4, 4, 512], F32, tag="ss")
                        for nn in range(4):
                            nc.tensor.matmul(
                                ss[:, nn, :], bi4[:],
                                sq[:, nn * 512:(nn + 1) * 512],
                                start=True, stop=True)
                        sn = np_.tile([4, NP], F32, tag="sn")
                        nc.scalar.sqrt(
                            sn[:].rearrange("p (a b) -> p a b", a=4), ss[:])
                        # bounce reshape: [4, 2048] -> [128, 64]
                        nc.sync.dma_start(
                            out=scr_a[si].rearrange("h a p -> h (a p)"),
                            in_=sn[:])
                        rsh = np_.tile([128, 64], F32, tag="rsh")
                        nc.sync.dma_start(
                            out=rsh[:],
                            in_=scr_a[si].rearrange("h a p -> p (h a)"))
                        rin = np_.tile([128, 64], F32, tag="rin")
                        nc.vector.reciprocal(rin[:], rsh[:])
                        nc.sync.dma_start(
                            out=scr_b[si].rearrange("h a p -> p (h a)"),
                            in_=rin[:])
                        inkb = np_.tile([128, NP], F32, tag="inkb")
                        for h4 in range(4):
                            bc = bass.AP(
                                tensor=scr_b[si].tensor,
                                offset=h4 * NP,
                                ap=[[0, 32], [1, NP]])
                            nc.sync.dma_start(
                                out=inkb[h4 * 32:(h4 + 1) * 32, :], in_=bc)
                        eng = nc.vector if cc == 1 else nc.gpsimd
                        eng.tensor_tensor(out=tens[cc][:], in0=tens[cc][:],
                                          in1=inkb[:], op=OP.mult)
                        si += 1

                # q norms -> inq (25/|q| per row), via PE transpose of [4,125]
                for s_i, qT in enumerate((qTc, qTr)):
                    for cc in range(2):
                        sq = np_.tile([128, R], F32, tag="sqq")
                        nc.vector.tensor_tensor(out=sq[:], in0=qT[cc][:],
                                                in1=qT[cc][:], op=OP.mult)
                        ssq = nps.tile([4, R], F32, tag="ssq")
                        nc.tensor.matmul(ssq[:], bi4[:], sq[:],
                                         start=True, stop=True)
                        snq = np_.tile([4, R], F32, tag="snq")
                        # sqrt(ss/625) = |q|/25
                        nc.scalar.activation(snq[:], ssq[:], AF.Sqrt,
                                             scale=1.0 / 625.0)
                        for t in range(2):
                            tp = nps.tile([128, 4], F32, tag="tq")
                            nc.tensor.transpose(
                                tp[0:RT, :], snq[:, t * RT:(t + 1) * RT],
                                ident[0:4, 0:4])
                            nc.vector.tensor_copy(
                                inq[0:RT, s_i * 16 + t * 8 + cc * 4:
                                    s_i * 16 + t * 8 + cc * 4 + 4],
                                tp[0:RT, :])
                nc.vector.reciprocal(inq[0:RT, :], inq[0:RT, :])

            # ---------------- Phase D: vv scores -> sim mask ----------------
            with tc.tile_pool(name="vvp", bufs=2, space="PSUM") as vvp:
                for t in range(2):
                    raw = vvp.tile([128, 4, 512], F32, tag="raw")
                    for nn in range(4):
                        for kk in range(2):
                            nc.tensor.matmul(
                                raw[0:RT, nn, :],
                                vT[kk][:, t * RT:t * RT + RT],
                                vT[kk][:, nn * 512:(nn + 1) * 512],
                                start=(kk == 0), stop=(kk == 1))
                    nc.vector.tensor_scalar(
                        out=smask[t][0:RT, :].rearrange("p (a b) -> p a b",
                                                        a=4),
                        in0=raw[0:RT, :, :], scalar1=6.0, scalar2=None,
                        op0=OP.is_gt)

            # ---------------- Phase E: attention ----------------
            with tc.tile_pool(name="att", bufs=2) as ap_, \
                 tc.tile_pool(name="scp", bufs=2, space="PSUM") as scp, \
                 tc.tile_pool(name="xup", bufs=2, space="PSUM") as xup:
                for t in range(2):
                    sim_t = ap_.tile([128, NP], F32, tag="sim")
                    icr = ap_.tile([128, 8], F32, tag="icr")
                    xu = [xup.tile([128, 128], F32, tag="xu", name="xu") for _ in range(2)]
                    for h in range(8):
                        cc, hh = h // 4, h % 4
                        b0 = 32 * hh
                        eC = ap_.tile([128, NP], F32, tag="eC")
                        eR = ap_.tile([128, NP], F32, tag="eR")
                        den = ap_.tile([128, 4], F32, tag="den")
                        for mi, (kT, qT, e_t, dof) in enumerate(
                                ((kTc, qTc, eC, 0), (kTr, qTr, eR, 2))):
                            for half in range(2):
                                sc = scp.tile([128, 2, 512], F32, tag="sc")
                                for nn in range(2):
                                    m0 = (half * 2 + nn) * 512
                                    nc.tensor.matmul(
                                        sc[0:RT, nn, :],
                                        qT[cc][b0:b0 + 32, t * RT:t * RT + RT],
                                        kT[cc][b0:b0 + 32, m0:m0 + 512],
                                        start=True, stop=True,
                                        tile_position=(b0, 0))
                                if half == 1:
                                    nc.vector.memset(
                                        sc[0:RT, 1, N - 1536:512], NEG)
                                nc.scalar.activation(
                                    e_t[0:RT, half * 1024:(half + 1) * 1024]
                                    .rearrange("p (a b) -> p a b", a=2),
                                    sc[0:RT, :, :], AF.Exp,
                                    scale=inq[0:RT, mi * 16 + t * 8 + h:
                                              mi * 16 + t * 8 + h + 1],
                                    accum_out=den[0:RT, dof + half:
                                                  dof + half + 1])
                        # denominators -> rho, icR2
                        dsum = ap_.tile([128, 2], F32, tag="dsum")
                        nc.vector.tensor_tensor(out=dsum[0:RT, 0:1],
                                                in0=den[0:RT, 0:1],
                                                in1=den[0:RT, 1:2], op=OP.add)
                        nc.vector.tensor_tensor(out=dsum[0:RT, 1:2],
                                                in0=den[0:RT, 2:3],
                                                in1=den[0:RT, 3:4], op=OP.add)
                        rden = ap_.tile([128, 2], F32, tag="rden")
                        nc.vector.reciprocal(rden[0:RT, :], dsum[0:RT, :])
                        rho = ap_.tile([128, 1], F32, tag="rho")
                        nc.vector.tensor_tensor(out=rho[0:RT, :],
                                                in0=dsum[0:RT, 1:2],
                                                in1=rden[0:RT, 0:1],
                                                op=OP.mult)
                        nc.vector.tensor_scalar(
                            out=icr[0:RT, h:h + 1], in0=rden[0:RT, 1:2],
                            scalar1=0.5, scalar2=None, op0=OP.mult)
                        # u = rho*eC + eR  (bf16)
                        u_bf = ap_.tile([128, NP], BF16, tag="u")
                        nc.vector.scalar_tensor_tensor(
                            out=u_bf[0:RT, :], in0=eC[0:RT, :],
                            scalar=rho[0:RT, :], in1=eR[0:RT, :],
                            op0=OP.mult, op1=OP.add)
                        # block mask strip
                        st = 0 if t == 0 else 120
                        nc.vector.tensor_tensor(
                            out=u_bf[0:RT, st:st + 130],
                            in0=u_bf[0:RT, st:st + 130],
                            in1=mst[0:RT, t, :], op=OP.mult)
                        # sim += icR2 * u
                        seng = nc.vector
                        if h == 0:
                            seng.tensor_scalar(
                                out=sim_t[0:RT, :], in0=u_bf[0:RT, :],
                                scalar1=icr[0:RT, h:h + 1], scalar2=None,
                                op0=OP.mult)
                        else:
                            seng.scalar_tensor_tensor(
                                out=sim_t[0:RT, :], in0=u_bf[0:RT, :],
                                scalar=icr[0:RT, h:h + 1], in1=sim_t[0:RT, :],
                                op0=OP.mult, op1=OP.add)
                        # transpose u via DMA xbar, then attn @ v
                        uT = ap_.tile([128, 16, 128], BF16, tag="uT")
                        nc.sync.dma_start_transpose(out=uT[:], in_=u_bf[:])
                        for j in range(16):
                            nc.tensor.matmul(
                                xu[cc][b0:b0 + 32, 0:RT],
                                v_bf[:, j, h * 32:h * 32 + 32],
                                uT[:, j, 0:RT],
                                start=(j == 0), stop=(j == 15),
                                tile_position=(0, b0))
                    # finalize x for this rowtile
                    x_sb = ap_.tile([128, C], F32, tag="xsb")
                    for cc in range(2):
                        xs = ap_.tile([128, 128], F32, tag="xs")
                        nc.vector.tensor_copy(xs[:, 0:RT], xu[cc][:, 0:RT])
                        xt = xup.tile([128, 128], F32, tag="xt")
                        nc.tensor.transpose(xt[0:RT, :], xs[:, 0:RT].rearrange(
                            "p r -> p r"), ident[:])
                        for hh in range(4):
                            h = cc * 4 + hh
                            nc.vector.tensor_scalar(
                                out=x_sb[0:RT, h * 32:h * 32 + 32],
                                in0=xt[0:RT, hh * 32:hh * 32 + 32],
                                scalar1=icr[0:RT, h:h + 1], scalar2=None,
                                op0=OP.mult)
                    nc.sync.dma_start(out=d_xout[t * RT:(t + 1) * RT, 0:C],
                                      in_=x_sb[0:RT, :])
                    # finalize sim_round2 for this rowtile
                    es = ap_.tile([128, NP], F32, tag="esm", name="es")
                    d1 = ap_.tile([128, 1], F32, tag="d1")
                    nc.scalar.activation(es[0:RT, :], sim_t[0:RT, :], AF.Exp,
                                         accum_out=d1[0:RT, :])
                    em = ap_.tile([128, NP], F32, tag="esm", name="em")
                    s_ = ap_.tile([128, 1], F32, tag="s_")
                    nc.vector.scalar_tensor_tensor(
                        out=em[0:RT, :], in0=es[0:RT, :], scalar=1.0,
                        in1=smask[t][0:RT, :], op0=OP.mult, op1=OP.mult,
                        accum_out=s_[0:RT, :])
                    dd = ap_.tile([128, 1], F32, tag="dd")
                    nc.vector.scalar_tensor_tensor(
                        out=dd[0:RT, :], in0=d1[0:RT, :], scalar=1e-8,
                        in1=s_[0:RT, :], op0=OP.mult, op1=OP.add)
                    rdd = ap_.tile([128, 1], F32, tag="rdd")
                    nc.vector.reciprocal(rdd[0:RT, :], dd[0:RT, :])
                    simo = ap_.tile([128, NP], F32, tag="esm", name="simo")
                    seng2 = nc.vector
                    seng2.tensor_scalar(out=simo[0:RT, :], in0=em[0:RT, :],
                                        scalar1=rdd[0:RT, :], scalar2=None,
                                        op0=OP.mult)
                    nc.sync.dma_start(out=d_sim[t * RT:(t + 1) * RT, :],
                                      in_=simo[0:RT, 0:N])
                # x_ori columns straight from v_nat rows 0..250
                nc.sync.dma_start(out=d_xout[0:128, C:2 * C],
                                  in_=v_nat[:, 0, :])
                nc.sync.dma_start(out=d_xout[128:R, C:2 * C],
                                  in_=v_nat[0:R - 128, 1, :])

    nc.compile()
    return nc


_NC_CACHE = None
_last_in_maps = None


def kernel(x_cls, x_reg, W_qkv_cls, W_qkv_reg):
    global _NC_CACHE
    x_cls = np.asarray(x_cls)
    x_reg = np.asarray(x_reg)
    W_qkv_cls = np.ascontiguousarray(np.asarray(W_qkv_cls), dtype=np.float32)
    W_qkv_reg = np.ascontiguousarray(np.asarray(W_qkv_reg), dtype=np.float32)
    xc0 = x_cls[0].astype(np.float32)
    xr0 = x_reg[0].astype(np.float32)

    bi4 = np.zeros((128, 4), dtype=np.float32)
    for h4 in range(4):
        bi4[h4 * 32:(h4 + 1) * 32, h4] = 1.0

    # mask strip (core-independent thanks to rotation; 250 % 10 == 0)
    mst = np.ones((2, RT, 130), dtype=np.float32)
    strip0 = [0, 120]
    for t in range(2):
        for i in range(RT):
            row = t * RT + i
            blk = (row // 10) * 10
            for j in range(blk, blk + 9):
                if j != row:
                    mst[t, i, j - strip0[t]] = 0.0

    if _NC_CACHE is None:
        _NC_CACHE = _build_program()
    nc = _NC_CACHE

    in_maps = []
    for core in range(NCORES):
        r0 = R * core
        xc = np.roll(xc0, -r0, axis=0)
        xr = np.roll(xr0, -r0, axis=0)
        xcp = np.zeros((NP, C), dtype=np.float32)
        xcp[0:N] = xc
        xrp = np.zeros((NP, C), dtype=np.float32)
        xrp[0:N] = xr
        in_maps.append({
            "xc": xcp, "xr": xrp, "wc": W_qkv_cls, "wr": W_qkv_reg,
            "bi4": bi4, "mst": mst,
        })

    global _last_in_maps
    _last_in_maps = in_maps
    res = run_bass_kernel_spmd(nc, in_maps, core_ids=list(range(NCORES)))

    x_out = np.zeros((1, N, 2 * C), dtype=np.float32)
    sim = np.zeros((N, N), dtype=np.float32)
    for core in range(NCORES):
        r0 = R * core
        x_out[0, r0:r0 + R, :] = res.results[core]["xout"]
        sim[r0:r0 + R, :] = np.roll(res.results[core]["simout"], r0, axis=1)
    return (x_out, sim)


if __name__ == "__main__":
    import reference
    ins = reference.setup_inputs()
    out = kernel(**{k: np.asarray(v) for k, v in ins.items()})
    print("ok", out[0].shape, out[1].shape)
